# revision 24
# baseline (speedup 1.0000x reference)
"""Trainium2 Bass kernel for nn_LocalModel_76527727280750 (sparse_attention).

8-core SPMD: head-parallel attention (core c owns head c, both batches) +
token-parallel LayerNorm/FFN. Each core owns 256 tokens of EACH batch
(batch b tokens [c*256,(c+1)*256)), so attention output resharding is one
AllToAll per batch. Emission order pipelines per batch so every collective
overlaps PE work:
  stacks(0) -> attn(0)+A2A(0) -> stacks(1) -> attn(1)+A2A(1)
  -> ffn(0)+AG(0) -> ffn(1)+AG(1) -> next layer (qkv b0 needs only AG(0)).
Next-layer input is replicated via two per-batch AllGathers (transposed,
bf16).

Self-contained: hardcodes all shapes; host does the embedding gather,
weight slicing/packing, and the tiny final reduction.
"""

import sys

for _p in ("/opt/trn_rl_repo",):
    if _p not in sys.path:
        sys.path.append(_p)

import numpy as np
import ml_dtypes

import concourse.bass as bass  # noqa: F401  (registers types)
import concourse.mybir as mybir
import concourse.tile as tile
from concourse import bacc
from concourse.bass_utils import run_bass_kernel_spmd
from concourse.masks import make_identity

# ---- model dims (hardcoded from the problem spec) ----
NC = 8
B, S, E, H, W, HID, V, OUT, L = 2, 2048, 512, 8, 5, 2048, 32000, 6, 6
DH = E // H            # 64
SCALE = DH ** -0.5     # 0.125
PAD = (W - 1) // 2     # 2
SK = S - W + 1         # 2044
BS = B * S             # 4096
CHUNK = BS // NC       # 512 tokens per core (256 from each batch)
HALF = CHUNK // 2      # 256
NT = (SK + 127) // 128  # 16 t-blocks (last = 124 wide)
NHB = HID // 128       # 16

f32 = mybir.dt.float32
bf16 = mybir.dt.bfloat16
AF = mybir.ActivationFunctionType
ALU = mybir.AluOpType


def _twidth(tb):
    return min(128, SK - tb * 128)


def build_nc(timing=False, trivial=False):
    ndev = 1 if timing else NC
    nc = bacc.Bacc("TRN2", target_bir_lowering=False, debug=False,
                   enable_asserts=False, num_devices=ndev)

    # ---------------- I/O ----------------
    # xsrc layout (per batch, AG-compatible): row = c*512 + e;
    # col j in [0,256): batch-b token c*256+j.
    xsrc_d = [nc.dram_tensor(f"xsrc{b}", [BS, HALF], bf16,
                             kind="ExternalInput") for b in range(B)]
    qkw_d = nc.dram_tensor("qkw", [128, 4, 128], bf16, kind="ExternalInput")
    vw_d = nc.dram_tensor("vw", [128, 4, DH], bf16, kind="ExternalInput")
    qkb_d = nc.dram_tensor("qkb", [128, 1], f32, kind="ExternalInput")
    vb_d = nc.dram_tensor("vb", [DH, 1], f32, kind="ExternalInput")
    fc1w_d = nc.dram_tensor("fc1w", [128, 4, HID], bf16, kind="ExternalInput")
    fc1b_d = nc.dram_tensor("fc1b", [128, NHB], f32, kind="ExternalInput")
    fc2w_d = nc.dram_tensor("fc2w", [128, NHB, E], bf16, kind="ExternalInput")
    fc2b_d = nc.dram_tensor("fc2b", [E], f32, kind="ExternalInput")
    lnw_d = nc.dram_tensor("lnw", [E], f32, kind="ExternalInput")
    lnb_d = nc.dram_tensor("lnb", [E], f32, kind="ExternalInput")
    outw_d = nc.dram_tensor("outw", [OUT, HALF, E], f32, kind="ExternalInput")
    headp_d = nc.dram_tensor("headp", [4, 128, OUT], f32, kind="ExternalOutput")

    with tile.TileContext(nc) as tc:
        with (
            tc.tile_pool(name="const", bufs=1) as cst,
            tc.tile_pool(name="xt", bufs=3) as xt_pool,
            tc.tile_pool(name="stk", bufs=1) as stk,
            tc.tile_pool(name="work", bufs=2) as work,
            tc.tile_pool(name="pt", bufs=4) as ptp,
            tc.tile_pool(name="small", bufs=4) as small,
            tc.tile_pool(name="ps_mm", bufs=4, space="PSUM") as ps_mm,
            tc.tile_pool(name="ps_o", bufs=2, space="PSUM") as ps_o,
            tc.tile_pool(name="ps_tr", bufs=2, space="PSUM") as ps_tr,
            tc.tile_pool(name="dram", bufs=2, space="DRAM") as dram,
        ):
            # -------- persistent constants --------
            qkw_sb = cst.tile([128, 4, 128], bf16)
            nc.sync.dma_start(qkw_sb[:], qkw_d[:])
            vw_sb = cst.tile([128, 4, DH], bf16)
            nc.sync.dma_start(vw_sb[:], vw_d[:])
            qkb_sb = cst.tile([128, 1], f32)
            nc.sync.dma_start(qkb_sb[:], qkb_d[:])
            vb_sb = cst.tile([DH, 1], f32)
            nc.sync.dma_start(vb_sb[:], vb_d[:])
            fc1w_sb = cst.tile([128, 4, HID], bf16)
            nc.sync.dma_start(fc1w_sb[:], fc1w_d[:])
            fc1b_sb = cst.tile([128, NHB], f32)
            nc.sync.dma_start(fc1b_sb[:], fc1b_d[:])
            fc2w_sb = cst.tile([128, NHB, E], bf16)
            nc.sync.dma_start(fc2w_sb[:], fc2w_d[:])
            fc2b_bc = cst.tile([128, E], f32)
            nc.sync.dma_start(fc2b_bc[:], fc2b_d.ap()[None, :].to_broadcast([128, E]))
            lnw_bc = cst.tile([128, E], f32)
            nc.sync.dma_start(lnw_bc[:], lnw_d.ap()[None, :].to_broadcast([128, E]))
            lnb_bc = cst.tile([128, E], f32)
            nc.sync.dma_start(lnb_bc[:], lnb_d.ap()[None, :].to_broadcast([128, E]))
            ident = cst.tile([128, 128], f32)
            make_identity(nc, ident[:])
            eps_sb = cst.tile([128, 1], f32)
            nc.vector.memset(eps_sb[:], 1e-5)

            # per-batch source of x^T for the current layer
            xsrc_holder = [xsrc_d[0].ap(), xsrc_d[1].ap()]

            def load_xtg(b, g):
                """[128,4,2,256] bf16 x^T tile: all E rows, batch-b tokens
                [g*512,(g+1)*512) (core-chunks 2g/2g+1), two 3D-AP DMAs.
                Slice [:, ec, :, :] is a [128,512] matmul rhs."""
                xtg = xt_pool.tile([128, 4, 2, HALF], bf16, tag="xt", name="xtg")
                for piece in range(2):
                    cc = 2 * g + piece
                    src = xsrc_holder[b][cc * 512:(cc + 1) * 512, 0:HALF]
                    nc.sync.dma_start(
                        xtg[:, :, piece, :],
                        src.rearrange("(ec p) j -> p ec j", ec=4))
                return xtg

            def layer_norm(xap):
                # var = E[x^2] - mu^2; apply as (x + mneg) * rstd in one pass
                mneg = small.tile([128, 1], f32, tag="mneg", name="mneg")
                nc.vector.reduce_sum(mneg[:], xap, axis=mybir.AxisListType.X)
                sq = work.tile([128, E], bf16, tag="sq", bufs=1, name="sq")
                ss = small.tile([128, 1], f32, tag="ss", name="ss")
                nc.vector.tensor_mul(sq[:], xap, xap)
                nc.vector.reduce_sum(ss[:], sq[:], axis=mybir.AxisListType.X)
                nc.vector.tensor_scalar_mul(mneg[:], mneg[:], -1.0 / E)
                mu2 = small.tile([128, 1], f32, tag="mu2", name="mu2")
                nc.vector.tensor_mul(mu2[:], mneg[:], mneg[:])
                var = small.tile([128, 1], f32, tag="var", name="var")
                nc.vector.tensor_scalar(var[:], ss[:], 1.0 / E, None, ALU.mult)
                nc.vector.tensor_sub(var[:], var[:], mu2[:])
                sd = small.tile([128, 1], f32, tag="sd", name="sd")
                nc.scalar.activation(sd[:], var[:], AF.Sqrt, bias=eps_sb[:])
                rs = small.tile([128, 1], f32, tag="rs", name="rs")
                nc.vector.reciprocal(rs[:], sd[:])
                nc.vector.tensor_scalar(xap, xap, mneg[:], rs[:], ALU.add, ALU.mult)
                if not trivial:
                    nc.vector.tensor_mul(xap, xap, lnw_bc[:])
                    nc.vector.tensor_add(xap, xap, lnb_bc[:])

            for l in range(L):
                qs, ks, vaug = {}, {}, {}

                def build_stacks(b):
                    """q/k/v projections + shifted stacks for batch b."""
                    qs0 = stk.tile([128, S], bf16, tag=f"qs0_{b}", name="qs0")
                    qs1 = stk.tile([128, S], bf16, tag=f"qs1_{b}", name="qs1")
                    qs2 = stk.tile([128, S], bf16, tag=f"qs2_{b}", name="qs2")
                    ks0 = stk.tile([128, S], bf16, tag=f"ks0_{b}", name="ks0")
                    ks1 = stk.tile([128, S], bf16, tag=f"ks1_{b}", name="ks1")
                    ks2 = stk.tile([128, S], bf16, tag=f"ks2_{b}", name="ks2")
                    # only the shift edges are never written by the copies below
                    for t in (qs0, qs1, qs2):
                        nc.vector.memset(t[:, 0:2], 0.0)
                        nc.vector.memset(t[:, S - 2:S], 0.0)
                    qs[b] = (qs0, qs1, qs2)
                    ks[b] = (ks0, ks1, ks2)

                    vT = stk.tile([DH, S], f32, tag=f"vT_{b}", name="vT")
                    vs = stk.tile([DH + 1, S], f32, tag=f"vs_{b}", name="vs")
                    nc.vector.memset(vs[DH:DH + 1, 0:SK], 1.0)

                    for g in range(4):
                        qk_ps3 = ps_mm.tile([128, 512], f32, tag="mmps",
                                            name="qk_ps")
                        v_ps3 = ps_mm.tile([128, 512], f32, tag="mmps",
                                           name="v_ps")
                        qk_ps = qk_ps3[:]
                        v_ps = v_ps3[:]
                        xtg = load_xtg(b, g)
                        for ec in range(4):
                            xt = xtg[:, ec, :, :]
                            nc.tensor.matmul(qk_ps, qkw_sb[:, ec, :], xt,
                                             start=(ec == 0), stop=(ec == 3))
                            nc.tensor.matmul(v_ps[0:DH, :], vw_sb[:, ec, :], xt,
                                             start=(ec == 0), stop=(ec == 3))
                        qk_sb = work.tile([128, 512], bf16, tag="qksb", bufs=2,
                                          name="qk_sb")
                        if trivial:
                            nc.vector.tensor_copy(qk_sb[:], qk_ps)
                            nc.vector.tensor_copy(vT[:, g * 512:(g + 1) * 512],
                                                  v_ps[0:DH, :])
                        else:
                            nc.vector.tensor_scalar_add(qk_sb[:], qk_ps, qkb_sb[:])
                            nc.vector.tensor_scalar_add(vT[:, g * 512:(g + 1) * 512],
                                                        v_ps[0:DH, :], vb_sb[:])
                        # shifted copies into stacks:
                        # q rows hold qT[:, s+j-2] (dest = src + 2-j);
                        # k rows hold kT[:, t+j]   (dest = src - j).
                        # j4 duplicated into rows 64:128 for row-group packing.
                        qdst = [(qs0, 0, 2), (qs0, 64, 1), (qs1, 0, 0), (qs1, 64, -1),
                                (qs2, 0, -2), (qs2, 64, -2)]
                        kdst = [(ks0, 0, 0), (ks0, 64, -1), (ks1, 0, -2), (ks1, 64, -3),
                                (ks2, 0, -4), (ks2, 64, -4)]
                        for (srow, lim, lst) in ((0, S, qdst), (64, SK, kdst)):
                            for (dstt, drow, off) in lst:
                                lo = max(0, g * 512 + off)
                                hi = min(lim, g * 512 + 512 + off)
                                if hi <= lo:
                                    continue
                                nc.vector.tensor_copy(
                                    dstt[drow:drow + 64, lo:hi],
                                    qk_sb[srow:srow + 64,
                                          lo - off - g * 512:hi - off - g * 512])

                        # windowed v-sum for the range whose 5-wide window is
                        # fully covered by vT chunks written so far; chunked
                        # per g so the DVE adds overlap the qkv matmuls.
                        vlo = max(0, g * 512 - 4)
                        vhi = min(SK, (g + 1) * 512 - 4) if g < 3 else SK
                        nc.gpsimd.tensor_add(vs[0:DH, vlo:vhi], vT[:, vlo:vhi],
                                             vT[:, vlo + 1:vhi + 1])
                        nc.gpsimd.tensor_add(vs[0:DH, vlo:vhi], vs[0:DH, vlo:vhi],
                                             vT[:, vlo + 2:vhi + 2])
                        nc.gpsimd.tensor_add(vs[0:DH, vlo:vhi], vs[0:DH, vlo:vhi],
                                             vT[:, vlo + 3:vhi + 3])
                        nc.gpsimd.tensor_add(vs[0:DH, vlo:vhi], vs[0:DH, vlo:vhi],
                                             vT[:, vlo + 4:vhi + 4])

                    # va transposes are emitted inside attention()'s gp==0
                    # tb loop so they fill the PE stream instead of stalling it
                    va = stk.tile([128, NT, DH + 1], bf16, tag=f"vaug_{b}", name="va")
                    vaug[b] = (va, vs)

                # per-layer tiles for the LN/FFN pipeline
                y_all = work.tile([128, 4, E], f32, tag="yall", bufs=1, name="y_all")
                yT_sb = work.tile([128, 4, 512], bf16, tag="yT", bufs=1, name="yT_sb")
                hT_sb = work.tile([128, NHB, 512], bf16, tag="hT", bufs=1,
                                  name="hT_sb")
                xn_all = work.tile([128, 4, E], f32, tag="xn", bufs=1, name="xn_all")
                if l < L - 1:
                    xTc_sb = work.tile([128, 4, 512], bf16, tag="xTc", bufs=1,
                                       name="xTc_sb")
                a2a_outs = {}

                def attention(b):
                    qs0, qs1, qs2 = qs[b]
                    ks0, ks1, ks2 = ks[b]
                    va, vs = vaug[b]
                    a2a_in = dram.tile([S, DH], f32, tag=f"a2a_in{b}", name="a2a_in")
                    for gp in range(2):
                        oT_ps2 = [ps_o.tile([DH + 1, 512], f32, tag="ops",
                                            name=f"oT_ps_{gi}")
                                  for gi in range(2)]
                        for tb in range(NT):
                            tw = _twidth(tb)
                            s_ps2 = [ps_mm.tile([128, 512], f32, tag="mmps",
                                                name=f"s_ps_{gi}")
                                     for gi in range(2)]
                            for gi in range(2):
                                g = gp * 2 + gi
                                sl = s_ps2[gi][0:tw, :]
                                nc.tensor.matmul(sl, ks0[:, tb * 128:tb * 128 + tw],
                                                 qs0[:, g * 512:(g + 1) * 512],
                                                 start=True, stop=False)
                                nc.tensor.matmul(sl, ks1[:, tb * 128:tb * 128 + tw],
                                                 qs1[:, g * 512:(g + 1) * 512],
                                                 start=False, stop=False)
                                # K=64 j4 chunk; gi=1 uses rows 64:128 so the two
                                # matmuls pack into disjoint PE row groups.
                                rlo = gi * 64
                                nc.tensor.matmul(
                                    sl, ks2[rlo:rlo + 64, tb * 128:tb * 128 + tw],
                                    qs2[rlo:rlo + 64, g * 512:(g + 1) * 512],
                                    start=False, stop=True)
                            if gp == 0:
                                # v_aug transpose for this t-block, interleaved
                                # with the score matmuls to keep PE dense
                                trp = ps_tr.tile([128, 128], f32, tag="trps",
                                                 name="trp")
                                nc.tensor.transpose(trp[0:tw, 0:DH + 1],
                                                    vs[:, tb * 128:tb * 128 + tw],
                                                    ident[0:DH + 1, 0:DH + 1])
                                nc.any.tensor_copy(va[0:tw, tb, :],
                                                   trp[0:tw, 0:DH + 1])
                            for gi in range(2):
                                pt = ptp.tile([128, 512], bf16, tag="pt", name="pt")
                                nc.scalar.activation(pt[0:tw, :], s_ps2[gi][0:tw, :],
                                                     AF.Exp, scale=SCALE)
                                nc.tensor.matmul(oT_ps2[gi][:], va[0:tw, tb, :],
                                                 pt[0:tw, :],
                                                 start=(tb == 0), stop=(tb == NT - 1))
                        for gi in range(2):
                            g = gp * 2 + gi
                            oT_sb = work.tile([DH + 1, 512], f32, tag="otsb",
                                              name="oT_sb")
                            nc.any.tensor_copy(oT_sb[:], oT_ps2[gi][:])
                            o_st = small.tile([128, 4, DH], f32, tag="ost",
                                              name="o_st")
                            for tt in range(4):
                                trp = ps_tr.tile([128, 128], f32, tag="trps",
                                                 name="trp")
                                nc.tensor.transpose(trp[0:128, 0:DH + 1],
                                                    oT_sb[:, tt * 128:(tt + 1) * 128],
                                                    ident[0:DH + 1, 0:DH + 1])
                                rcp = small.tile([128, 1], f32, tag="rcp", name="rcp")
                                nc.vector.reciprocal(rcp[:], trp[:, DH:DH + 1])
                                nc.vector.tensor_scalar_mul(o_st[:, tt, :],
                                                            trp[:, 0:DH], rcp[:])
                            nc.sync.dma_start(
                                a2a_in[g * 512:(g + 1) * 512, :].rearrange(
                                    "(tt p) d -> p tt d", tt=4),
                                o_st[:])
                    # reshard batch b: head-split -> 256-token-split
                    a2a_out = dram.tile([S, DH], f32, tag=f"a2a_out{b}",
                                        name="a2a_out")
                    if timing:
                        nc.sync.dma_start(a2a_out[0:8, :], a2a_in[0:8, :])
                    else:
                        nc.gpsimd.collective_compute(
                            "AllToAll", ALU.bypass,
                            replica_groups=[list(range(NC))],
                            ins=[a2a_in.opt()], outs=[a2a_out.opt()],
                        )
                    a2a_outs[b] = a2a_out

                def halfpipe(b):
                    """y gather + LN1 + yT + fc1 for batch b's 256-token half."""
                    a2a_src = a2a_outs[b][:].rearrange("(i r) d -> r i d", i=NC)
                    for ht in range(2):
                        tt = b * 2 + ht
                        yv = y_all[:, tt, :]
                        nc.sync.dma_start(
                            yv.rearrange("p (i d) -> p i d", d=DH),
                            a2a_src[ht * 128:(ht + 1) * 128, :, :])
                        layer_norm(yv)
                        for ec in range(4):
                            trp = ps_tr.tile([128, 128], f32, tag="trps", name="trp")
                            nc.tensor.transpose(trp[:], yv[:, ec * 128:(ec + 1) * 128],
                                                ident[:])
                            nc.any.tensor_copy(yT_sb[:, ec, tt * 128:(tt + 1) * 128],
                                               trp[:])
                    # fc1 for this half: 4 hid-blocks per 2-bank psum slot
                    for hq in range(NHB // 2):
                        h_ps = ps_mm.tile([128, 512], f32, tag="mmps", name="h_ps")
                        for hi in range(2):
                            hb = hq * 2 + hi
                            sl = h_ps[:, hi * 256:(hi + 1) * 256]
                            for ec in range(4):
                                nc.tensor.matmul(
                                    sl, fc1w_sb[:, ec, hb * 128:(hb + 1) * 128],
                                    yT_sb[:, ec, b * 256:(b + 1) * 256],
                                    start=(ec == 0), stop=(ec == 3))
                        for hi in range(2):
                            hb = hq * 2 + hi
                            if trivial:
                                nc.vector.tensor_scalar_max(
                                    hT_sb[:, hb, b * 256:(b + 1) * 256],
                                    h_ps[:, hi * 256:(hi + 1) * 256], 0.0)
                            else:
                                nc.vector.tensor_scalar(
                                    hT_sb[:, hb, b * 256:(b + 1) * 256],
                                    h_ps[:, hi * 256:(hi + 1) * 256],
                                    fc1b_sb[:, hb:hb + 1], 0.0, ALU.add, ALU.max)

                def tailpipe(b):
                    """fc2 + residual + LN2 (+head / transposes + AG-in DMA)."""
                    for ht in range(2):
                        tt = b * 2 + ht
                        x2_ps2 = ps_mm.tile([128, 512], f32, tag="mmps",
                                            name="x2_ps")
                        x2_ps = x2_ps2[:]
                        for hc in range(NHB):
                            nc.tensor.matmul(x2_ps,
                                             hT_sb[:, hc, tt * 128:(tt + 1) * 128],
                                             fc2w_sb[:, hc, :],
                                             start=(hc == 0), stop=(hc == NHB - 1))
                        xn = xn_all[:, tt, :]
                        nc.vector.tensor_add(xn, x2_ps, y_all[:, tt, :])
                        if not trivial:
                            nc.vector.tensor_add(xn, xn, fc2b_bc[:])
                        layer_norm(xn)
                        if l == L - 1:
                            acc = small.tile([128, OUT], f32, tag="acc", name="acc")
                            for o in range(OUT):
                                wro = work.tile([128, E], f32, tag="wro", bufs=2,
                                                name="wro")
                                nc.sync.dma_start(
                                    wro[:], outw_d[o, ht * 128:(ht + 1) * 128, :])
                                prod = work.tile([128, E], f32, tag="prod", bufs=2,
                                                 name="prod")
                                nc.vector.tensor_mul(prod[:], xn, wro[:])
                                nc.vector.reduce_sum(acc[:, o:o + 1], prod[:],
                                                     axis=mybir.AxisListType.X)
                            nc.sync.dma_start(headp_d[tt], acc[:])
                        else:
                            for ec in range(4):
                                trp = ps_tr.tile([128, 128], f32, tag="trps",
                                                 name="trp")
                                nc.tensor.transpose(trp[:],
                                                    xn[:, ec * 128:(ec + 1) * 128],
                                                    ident[:])
                                nc.any.tensor_copy(
                                    xTc_sb[:, ec, tt * 128:(tt + 1) * 128], trp[:])
                    if l < L - 1:
                        ag_in = dram.tile([E, HALF], bf16, tag=f"ag_in{b}",
                                          name="ag_in")
                        for ec in range(4):
                            nc.sync.dma_start(
                                ag_in[ec * 128:(ec + 1) * 128, :],
                                xTc_sb[:, ec, b * 256:(b + 1) * 256])
                        ag_out = dram.tile([BS, HALF], bf16, tag=f"ag_out{b}",
                                           addr_space="Local" if timing
                                           else "Shared",
                                           name="ag_out")
                        if timing:
                            nc.sync.dma_start(ag_out[0:8, :], ag_in[0:8, :])
                        else:
                            nc.gpsimd.collective_compute(
                                "AllGather", ALU.bypass,
                                replica_groups=[list(range(NC))],
                                ins=[ag_in.opt()], outs=[ag_out.opt()],
                            )
                        xsrc_holder[b] = ag_out[:]

                # emission order = per-engine program order: each batch's
                # attention runs back-to-back with the other batch's
                # collectives + LN/FFN so PE never waits on the network.
                build_stacks(0)
                attention(0)
                build_stacks(1)
                attention(1)
                halfpipe(0)
                tailpipe(0)
                halfpipe(1)
                tailpipe(1)

    nc.compile()
    return nc


# ---------------------------------------------------------------------------
# host side
# ---------------------------------------------------------------------------
_STATE: dict = {}


def _pos_encoding_np():
    pos = np.arange(S, dtype=np.float32)[:, None]
    div = np.exp(np.arange(0, E, 2, dtype=np.float32) * (-np.log(10000.0) / E))
    pe = np.zeros((S, E), np.float32)
    pe[:, 0::2] = np.sin(pos * div)
    pe[:, 1::2] = np.cos(pos * div)
    return pe


def _bf(x):
    return np.ascontiguousarray(np.asarray(x, np.float32).astype(ml_dtypes.bfloat16))


def _f32(x):
    return np.ascontiguousarray(np.asarray(x, np.float32))


def kernel(inputs, emb, ln_w, ln_b, q_w, q_b, k_w, k_b, v_w, v_b,
           fc1_w, fc1_b, fc2_w, fc2_b, out_w, out_b):
    idx = np.asarray(inputs)
    emb = _f32(emb)
    x0 = emb[idx.reshape(-1)] + np.tile(_pos_encoding_np(), (B, 1))  # [BS, E]
    # per-batch xsrc layout: row c*512+e; col j -> batch-b token c*256+j
    x0_b = x0.reshape(B, NC, HALF, E).transpose(0, 1, 3, 2)  # [B, NC, E, HALF]
    xsrc = [np.ascontiguousarray(x0_b[b].reshape(BS, HALF)) for b in range(B)]

    trivial = bool(
        np.all(np.asarray(ln_w, np.float32) == 1.0)
        and np.all(np.asarray(ln_b, np.float32) == 0.0)
        and np.all(np.asarray(q_b, np.float32) == 0.0)
        and np.all(np.asarray(k_b, np.float32) == 0.0)
        and np.all(np.asarray(v_b, np.float32) == 0.0)
        and np.all(np.asarray(fc1_b, np.float32) == 0.0)
        and np.all(np.asarray(fc2_b, np.float32) == 0.0))
    key = ("nc", trivial)
    if key not in _STATE:
        _STATE[key] = build_nc(trivial=trivial)
    nc = _STATE[key]

    q_w, k_w, v_w = _f32(q_w), _f32(k_w), _f32(v_w)
    fc1_w, fc2_w = _f32(fc1_w), _f32(fc2_w)
    out_w = _f32(out_w)
    Wr = out_w.reshape(S, E, OUT)

    fc1_pack = _bf(fc1_w.reshape(4, 128, HID).transpose(1, 0, 2))
    fc1b_pack = _f32(np.asarray(fc1_b, np.float32).reshape(NHB, 128).T)
    fc2_pack = _bf(fc2_w.reshape(NHB, 128, E).transpose(1, 0, 2))

    in_maps = []
    for c in range(NC):
        hs = slice(c * DH, (c + 1) * DH)
        qk = np.concatenate([q_w[:, hs], k_w[:, hs]], axis=1)  # [E, 128]
        in_maps.append({
            "xsrc0": _bf(xsrc[0]),
            "xsrc1": _bf(xsrc[1]),
            "qkw": _bf(qk.reshape(4, 128, 128).transpose(1, 0, 2)),
            "vw": _bf(v_w[:, hs].reshape(4, 128, DH).transpose(1, 0, 2)),
            "qkb": _f32(np.concatenate([np.asarray(q_b, np.float32)[hs],
                                        np.asarray(k_b, np.float32)[hs]])[:, None]),
            "vb": _f32(np.asarray(v_b, np.float32)[hs][:, None]),
            "fc1w": fc1_pack,
            "fc1b": fc1b_pack,
            "fc2w": fc2_pack,
            "fc2b": _f32(fc2_b),
            "lnw": _f32(ln_w),
            "lnb": _f32(ln_b),
            "outw": _f32(Wr[c * HALF:(c + 1) * HALF].transpose(2, 0, 1)),
        })

    res = run_bass_kernel_spmd(nc, in_maps, core_ids=list(range(NC)))
    _STATE["last_results"] = res

    out = np.zeros((B, OUT), np.float64)
    for c in range(NC):
        hp = res.results[c]["headp"]  # [4,128,OUT]; tt 0,1 -> batch0, 2,3 -> batch1
        out[0] += hp[0:2].sum(axis=(0, 1), dtype=np.float64)
        out[1] += hp[2:4].sum(axis=(0, 1), dtype=np.float64)
    out += np.asarray(out_b, np.float32)[None, :].astype(np.float64)
    return out.astype(np.float32)


# revision 25
# speedup vs baseline: 1.1817x; 1.1817x over previous
"""Trainium2 Bass kernel for nn_LocalModel_76527727280750 (sparse_attention).

8-core SPMD: head-parallel attention (core c owns head c, both batches) +
token-parallel LayerNorm/FFN. Each core owns 256 tokens of EACH batch
(batch b tokens [c*256,(c+1)*256)), so attention output resharding is one
AllToAll per batch. Emission order pipelines per batch so every collective
overlaps PE work:
  stacks(0) -> attn(0)+A2A(0) -> stacks(1) -> attn(1)+A2A(1)
  -> ffn(0)+AG(0) -> ffn(1)+AG(1) -> next layer (qkv b0 needs only AG(0)).
Next-layer input is replicated via two per-batch AllGathers (transposed,
bf16).

Self-contained: hardcodes all shapes; host does the embedding gather,
weight slicing/packing, and the tiny final reduction.
"""

import sys

for _p in ("/opt/trn_rl_repo",):
    if _p not in sys.path:
        sys.path.append(_p)

import numpy as np
import ml_dtypes

import concourse.bass as bass  # noqa: F401  (registers types)
import concourse.mybir as mybir
import concourse.tile as tile
from concourse import bacc
from concourse.bass_utils import run_bass_kernel_spmd
from concourse.masks import make_identity

# ---- model dims (hardcoded from the problem spec) ----
NC = 8
B, S, E, H, W, HID, V, OUT, L = 2, 2048, 512, 8, 5, 2048, 32000, 6, 6
DH = E // H            # 64
SCALE = DH ** -0.5     # 0.125
PAD = (W - 1) // 2     # 2
SK = S - W + 1         # 2044
BS = B * S             # 4096
CHUNK = BS // NC       # 512 tokens per core (256 from each batch)
HALF = CHUNK // 2      # 256
NT = (SK + 127) // 128  # 16 t-blocks (last = 124 wide)
NHB = HID // 128       # 16

f32 = mybir.dt.float32
bf16 = mybir.dt.bfloat16
AF = mybir.ActivationFunctionType
ALU = mybir.AluOpType


def _twidth(tb):
    return min(128, SK - tb * 128)


def build_nc(timing=False, trivial=False):
    ndev = 1 if timing else NC
    nc = bacc.Bacc("TRN2", target_bir_lowering=False, debug=False,
                   enable_asserts=False, num_devices=ndev)

    # ---------------- I/O ----------------
    # xsrc layout (per batch, AG-compatible): row = c*512 + e;
    # col j in [0,256): batch-b token c*256+j.
    xsrc_d = [nc.dram_tensor(f"xsrc{b}", [BS, HALF], bf16,
                             kind="ExternalInput") for b in range(B)]
    qkw_d = nc.dram_tensor("qkw", [128, 4, 128], bf16, kind="ExternalInput")
    vw_d = nc.dram_tensor("vw", [128, 4, DH], bf16, kind="ExternalInput")
    qkb_d = nc.dram_tensor("qkb", [128, 1], f32, kind="ExternalInput")
    vb_d = nc.dram_tensor("vb", [DH, 1], f32, kind="ExternalInput")
    fc1w_d = nc.dram_tensor("fc1w", [128, 4, HID], bf16, kind="ExternalInput")
    fc1b_d = nc.dram_tensor("fc1b", [128, NHB], f32, kind="ExternalInput")
    fc2w_d = nc.dram_tensor("fc2w", [128, NHB, E], bf16, kind="ExternalInput")
    fc2b_d = nc.dram_tensor("fc2b", [E], f32, kind="ExternalInput")
    lnw_d = nc.dram_tensor("lnw", [E], f32, kind="ExternalInput")
    lnb_d = nc.dram_tensor("lnb", [E], f32, kind="ExternalInput")
    outw_d = nc.dram_tensor("outw", [OUT, HALF, E], f32, kind="ExternalInput")
    headp_d = nc.dram_tensor("headp", [4, 128, OUT], f32, kind="ExternalOutput")

    with tile.TileContext(nc) as tc:
        with (
            tc.tile_pool(name="const", bufs=1) as cst,
            tc.tile_pool(name="xt", bufs=3) as xt_pool,
            tc.tile_pool(name="stk", bufs=1) as stk,
            tc.tile_pool(name="work", bufs=2) as work,
            tc.tile_pool(name="pt", bufs=4) as ptp,
            tc.tile_pool(name="small", bufs=4) as small,
            tc.tile_pool(name="ps_mm", bufs=4, space="PSUM") as ps_mm,
            tc.tile_pool(name="ps_o", bufs=2, space="PSUM") as ps_o,
            tc.tile_pool(name="ps_tr", bufs=2, space="PSUM") as ps_tr,
            tc.tile_pool(name="dram", bufs=2, space="DRAM") as dram,
        ):
            # -------- persistent constants --------
            qkw_sb = cst.tile([128, 4, 128], bf16)
            nc.sync.dma_start(qkw_sb[:], qkw_d[:])
            vw_sb = cst.tile([128, 4, DH], bf16)
            nc.sync.dma_start(vw_sb[:], vw_d[:])
            qkb_sb = cst.tile([128, 1], f32)
            nc.sync.dma_start(qkb_sb[:], qkb_d[:])
            vb_sb = cst.tile([DH, 1], f32)
            nc.sync.dma_start(vb_sb[:], vb_d[:])
            fc1w_sb = cst.tile([128, 4, HID], bf16)
            nc.sync.dma_start(fc1w_sb[:], fc1w_d[:])
            fc1b_sb = cst.tile([128, NHB], f32)
            nc.sync.dma_start(fc1b_sb[:], fc1b_d[:])
            fc2w_sb = cst.tile([128, NHB, E], bf16)
            nc.sync.dma_start(fc2w_sb[:], fc2w_d[:])
            fc2b_bc = cst.tile([128, E], f32)
            nc.sync.dma_start(fc2b_bc[:], fc2b_d.ap()[None, :].to_broadcast([128, E]))
            lnw_bc = cst.tile([128, E], f32)
            nc.sync.dma_start(lnw_bc[:], lnw_d.ap()[None, :].to_broadcast([128, E]))
            lnb_bc = cst.tile([128, E], f32)
            nc.sync.dma_start(lnb_bc[:], lnb_d.ap()[None, :].to_broadcast([128, E]))
            ident = cst.tile([128, 128], f32)
            make_identity(nc, ident[:])
            eps_sb = cst.tile([128, 1], f32)
            nc.vector.memset(eps_sb[:], 1e-5)

            # per-batch source of x^T for the current layer
            xsrc_holder = [xsrc_d[0].ap(), xsrc_d[1].ap()]

            def load_xtg(b, g):
                """[128,4,2,256] bf16 x^T tile: all E rows, batch-b tokens
                [g*512,(g+1)*512) (core-chunks 2g/2g+1), two 3D-AP DMAs.
                Slice [:, ec, :, :] is a [128,512] matmul rhs."""
                xtg = xt_pool.tile([128, 4, 2, HALF], bf16, tag="xt", name="xtg")
                for piece in range(2):
                    cc = 2 * g + piece
                    src = xsrc_holder[b][cc * 512:(cc + 1) * 512, 0:HALF]
                    nc.sync.dma_start(
                        xtg[:, :, piece, :],
                        src.rearrange("(ec p) j -> p ec j", ec=4))
                return xtg

            def layer_norm(xap):
                # var = E[x^2] - mu^2; apply as (x + mneg) * rstd in one pass
                mneg = small.tile([128, 1], f32, tag="mneg", name="mneg")
                nc.vector.reduce_sum(mneg[:], xap, axis=mybir.AxisListType.X)
                sq = work.tile([128, E], bf16, tag="sq", bufs=1, name="sq")
                ss = small.tile([128, 1], f32, tag="ss", name="ss")
                nc.vector.tensor_mul(sq[:], xap, xap)
                nc.vector.reduce_sum(ss[:], sq[:], axis=mybir.AxisListType.X)
                nc.vector.tensor_scalar_mul(mneg[:], mneg[:], -1.0 / E)
                mu2 = small.tile([128, 1], f32, tag="mu2", name="mu2")
                nc.vector.tensor_mul(mu2[:], mneg[:], mneg[:])
                var = small.tile([128, 1], f32, tag="var", name="var")
                nc.vector.tensor_scalar(var[:], ss[:], 1.0 / E, None, ALU.mult)
                nc.vector.tensor_sub(var[:], var[:], mu2[:])
                sd = small.tile([128, 1], f32, tag="sd", name="sd")
                nc.scalar.activation(sd[:], var[:], AF.Sqrt, bias=eps_sb[:])
                rs = small.tile([128, 1], f32, tag="rs", name="rs")
                nc.vector.reciprocal(rs[:], sd[:])
                nc.vector.tensor_scalar(xap, xap, mneg[:], rs[:], ALU.add, ALU.mult)
                if not trivial:
                    nc.vector.tensor_mul(xap, xap, lnw_bc[:])
                    nc.vector.tensor_add(xap, xap, lnb_bc[:])

            for l in range(L):
                qs, ks, vaug = {}, {}, {}

                def build_stacks(b):
                    """q/k/v projections + shifted stacks for batch b."""
                    qs0 = stk.tile([128, S], bf16, tag=f"qs0_{b}", name="qs0")
                    qs1 = stk.tile([128, S], bf16, tag=f"qs1_{b}", name="qs1")
                    qs2 = stk.tile([128, S], bf16, tag=f"qs2_{b}", name="qs2")
                    ks0 = stk.tile([128, S], bf16, tag=f"ks0_{b}", name="ks0")
                    ks1 = stk.tile([128, S], bf16, tag=f"ks1_{b}", name="ks1")
                    ks2 = stk.tile([128, S], bf16, tag=f"ks2_{b}", name="ks2")
                    # only the shift edges are never written by the copies below
                    for t in (qs0, qs1, qs2):
                        nc.vector.memset(t[:, 0:2], 0.0)
                        nc.vector.memset(t[:, S - 2:S], 0.0)
                    qs[b] = (qs0, qs1, qs2)
                    ks[b] = (ks0, ks1, ks2)

                    vT = stk.tile([DH, S], f32, tag=f"vT_{b}", name="vT")
                    vs = stk.tile([DH + 1, S], f32, tag=f"vs_{b}", name="vs")
                    nc.vector.memset(vs[DH:DH + 1, 0:SK], 1.0)

                    for g in range(4):
                        qk_ps3 = ps_mm.tile([128, 512], f32, tag="mmps",
                                            name="qk_ps")
                        v_ps3 = ps_mm.tile([128, 512], f32, tag="mmps",
                                           name="v_ps")
                        qk_ps = qk_ps3[:]
                        v_ps = v_ps3[:]
                        xtg = load_xtg(b, g)
                        for ec in range(4):
                            xt = xtg[:, ec, :, :]
                            nc.tensor.matmul(qk_ps, qkw_sb[:, ec, :], xt,
                                             start=(ec == 0), stop=(ec == 3))
                            nc.tensor.matmul(v_ps[0:DH, :], vw_sb[:, ec, :], xt,
                                             start=(ec == 0), stop=(ec == 3))
                        qk_sb = work.tile([128, 512], bf16, tag="qksb", bufs=2,
                                          name="qk_sb")
                        if trivial:
                            nc.vector.tensor_copy(qk_sb[:], qk_ps)
                            nc.vector.tensor_copy(vT[:, g * 512:(g + 1) * 512],
                                                  v_ps[0:DH, :])
                        else:
                            nc.vector.tensor_scalar_add(qk_sb[:], qk_ps, qkb_sb[:])
                            nc.vector.tensor_scalar_add(vT[:, g * 512:(g + 1) * 512],
                                                        v_ps[0:DH, :], vb_sb[:])
                        # shifted copies into stacks:
                        # q rows hold qT[:, s+j-2] (dest = src + 2-j);
                        # k rows hold kT[:, t+j]   (dest = src - j).
                        # j4 duplicated into rows 64:128 for row-group packing.
                        qdst = [(qs0, 0, 2), (qs0, 64, 1), (qs1, 0, 0), (qs1, 64, -1),
                                (qs2, 0, -2), (qs2, 64, -2)]
                        kdst = [(ks0, 0, 0), (ks0, 64, -1), (ks1, 0, -2), (ks1, 64, -3),
                                (ks2, 0, -4), (ks2, 64, -4)]
                        for (srow, lim, lst) in ((0, S, qdst), (64, SK, kdst)):
                            for (dstt, drow, off) in lst:
                                lo = max(0, g * 512 + off)
                                hi = min(lim, g * 512 + 512 + off)
                                if hi <= lo:
                                    continue
                                nc.vector.tensor_copy(
                                    dstt[drow:drow + 64, lo:hi],
                                    qk_sb[srow:srow + 64,
                                          lo - off - g * 512:hi - off - g * 512])

                        # windowed v-sum for the range whose 5-wide window is
                        # fully covered by vT chunks written so far; chunked
                        # per g so the DVE adds overlap the qkv matmuls.
                        vlo = max(0, g * 512 - 4)
                        vhi = min(SK, (g + 1) * 512 - 4) if g < 3 else SK
                        nc.vector.tensor_add(vs[0:DH, vlo:vhi], vT[:, vlo:vhi],
                                             vT[:, vlo + 1:vhi + 1])
                        nc.vector.tensor_add(vs[0:DH, vlo:vhi], vs[0:DH, vlo:vhi],
                                             vT[:, vlo + 2:vhi + 2])
                        nc.vector.tensor_add(vs[0:DH, vlo:vhi], vs[0:DH, vlo:vhi],
                                             vT[:, vlo + 3:vhi + 3])
                        nc.vector.tensor_add(vs[0:DH, vlo:vhi], vs[0:DH, vlo:vhi],
                                             vT[:, vlo + 4:vhi + 4])

                    # va transposes are emitted inside attention()'s gp==0
                    # tb loop so they fill the PE stream instead of stalling it
                    va = stk.tile([128, NT, DH + 1], bf16, tag=f"vaug_{b}", name="va")
                    vaug[b] = (va, vs)

                # per-layer tiles for the LN/FFN pipeline
                y_all = work.tile([128, 4, E], f32, tag="yall", bufs=1, name="y_all")
                yT_sb = work.tile([128, 4, 512], bf16, tag="yT", bufs=1, name="yT_sb")
                hT_sb = work.tile([128, NHB, 512], bf16, tag="hT", bufs=1,
                                  name="hT_sb")
                xn_all = work.tile([128, 4, E], f32, tag="xn", bufs=1, name="xn_all")
                if l < L - 1:
                    xTc_sb = work.tile([128, 4, 512], bf16, tag="xTc", bufs=1,
                                       name="xTc_sb")
                a2a_outs = {}

                def attention(b):
                    qs0, qs1, qs2 = qs[b]
                    ks0, ks1, ks2 = ks[b]
                    va, vs = vaug[b]
                    a2a_in = dram.tile([S, DH], f32, tag=f"a2a_in{b}", name="a2a_in")
                    for gp in range(2):
                        oT_ps2 = [ps_o.tile([DH + 1, 512], f32, tag="ops",
                                            name=f"oT_ps_{gi}")
                                  for gi in range(2)]
                        for tb in range(NT):
                            tw = _twidth(tb)
                            s_ps2 = [ps_mm.tile([128, 512], f32, tag="mmps",
                                                name=f"s_ps_{gi}")
                                     for gi in range(2)]
                            for gi in range(2):
                                g = gp * 2 + gi
                                sl = s_ps2[gi][0:tw, :]
                                nc.tensor.matmul(sl, ks0[:, tb * 128:tb * 128 + tw],
                                                 qs0[:, g * 512:(g + 1) * 512],
                                                 start=True, stop=False)
                                nc.tensor.matmul(sl, ks1[:, tb * 128:tb * 128 + tw],
                                                 qs1[:, g * 512:(g + 1) * 512],
                                                 start=False, stop=False)
                                # K=64 j4 chunk; gi=1 uses rows 64:128 so the two
                                # matmuls pack into disjoint PE row groups.
                                rlo = gi * 64
                                nc.tensor.matmul(
                                    sl, ks2[rlo:rlo + 64, tb * 128:tb * 128 + tw],
                                    qs2[rlo:rlo + 64, g * 512:(g + 1) * 512],
                                    start=False, stop=True)
                            if gp == 0:
                                # v_aug transpose for this t-block, interleaved
                                # with the score matmuls to keep PE dense
                                trp = ps_tr.tile([128, 128], f32, tag="trps",
                                                 name="trp")
                                nc.tensor.transpose(trp[0:tw, 0:DH + 1],
                                                    vs[:, tb * 128:tb * 128 + tw],
                                                    ident[0:DH + 1, 0:DH + 1])
                                nc.any.tensor_copy(va[0:tw, tb, :],
                                                   trp[0:tw, 0:DH + 1])
                            for gi in range(2):
                                pt = ptp.tile([128, 512], bf16, tag="pt", name="pt")
                                nc.scalar.activation(pt[0:tw, :], s_ps2[gi][0:tw, :],
                                                     AF.Exp, scale=SCALE)
                                nc.tensor.matmul(oT_ps2[gi][:], va[0:tw, tb, :],
                                                 pt[0:tw, :],
                                                 start=(tb == 0), stop=(tb == NT - 1))
                        for gi in range(2):
                            g = gp * 2 + gi
                            oT_sb = work.tile([DH + 1, 512], f32, tag="otsb",
                                              name="oT_sb")
                            nc.any.tensor_copy(oT_sb[:], oT_ps2[gi][:])
                            o_st = small.tile([128, 4, DH], f32, tag="ost",
                                              name="o_st")
                            for tt in range(4):
                                trp = ps_tr.tile([128, 128], f32, tag="trps",
                                                 name="trp")
                                nc.tensor.transpose(trp[0:128, 0:DH + 1],
                                                    oT_sb[:, tt * 128:(tt + 1) * 128],
                                                    ident[0:DH + 1, 0:DH + 1])
                                rcp = small.tile([128, 1], f32, tag="rcp", name="rcp")
                                nc.vector.reciprocal(rcp[:], trp[:, DH:DH + 1])
                                nc.vector.tensor_scalar_mul(o_st[:, tt, :],
                                                            trp[:, 0:DH], rcp[:])
                            nc.sync.dma_start(
                                a2a_in[g * 512:(g + 1) * 512, :].rearrange(
                                    "(tt p) d -> p tt d", tt=4),
                                o_st[:])
                    # reshard batch b: head-split -> 256-token-split
                    a2a_out = dram.tile([S, DH], f32, tag=f"a2a_out{b}",
                                        name="a2a_out")
                    if timing:
                        nc.sync.dma_start(a2a_out[0:8, :], a2a_in[0:8, :])
                    else:
                        nc.gpsimd.collective_compute(
                            "AllToAll", ALU.bypass,
                            replica_groups=[list(range(NC))],
                            ins=[a2a_in.opt()], outs=[a2a_out.opt()],
                        )
                    a2a_outs[b] = a2a_out

                def halfpipe(b):
                    """y gather + LN1 + yT + fc1 for batch b's 256-token half."""
                    a2a_src = a2a_outs[b][:].rearrange("(i r) d -> r i d", i=NC)
                    for ht in range(2):
                        tt = b * 2 + ht
                        yv = y_all[:, tt, :]
                        nc.sync.dma_start(
                            yv.rearrange("p (i d) -> p i d", d=DH),
                            a2a_src[ht * 128:(ht + 1) * 128, :, :])
                        layer_norm(yv)
                        for ec in range(4):
                            trp = ps_tr.tile([128, 128], f32, tag="trps", name="trp")
                            nc.tensor.transpose(trp[:], yv[:, ec * 128:(ec + 1) * 128],
                                                ident[:])
                            nc.any.tensor_copy(yT_sb[:, ec, tt * 128:(tt + 1) * 128],
                                               trp[:])
                    # fc1 for this half: 4 hid-blocks per 2-bank psum slot
                    for hq in range(NHB // 2):
                        h_ps = ps_mm.tile([128, 512], f32, tag="mmps", name="h_ps")
                        for hi in range(2):
                            hb = hq * 2 + hi
                            sl = h_ps[:, hi * 256:(hi + 1) * 256]
                            for ec in range(4):
                                nc.tensor.matmul(
                                    sl, fc1w_sb[:, ec, hb * 128:(hb + 1) * 128],
                                    yT_sb[:, ec, b * 256:(b + 1) * 256],
                                    start=(ec == 0), stop=(ec == 3))
                        for hi in range(2):
                            hb = hq * 2 + hi
                            if trivial:
                                nc.vector.tensor_scalar_max(
                                    hT_sb[:, hb, b * 256:(b + 1) * 256],
                                    h_ps[:, hi * 256:(hi + 1) * 256], 0.0)
                            else:
                                nc.vector.tensor_scalar(
                                    hT_sb[:, hb, b * 256:(b + 1) * 256],
                                    h_ps[:, hi * 256:(hi + 1) * 256],
                                    fc1b_sb[:, hb:hb + 1], 0.0, ALU.add, ALU.max)

                def tailpipe(b):
                    """fc2 + residual + LN2 (+head / transposes + AG-in DMA)."""
                    for ht in range(2):
                        tt = b * 2 + ht
                        x2_ps2 = ps_mm.tile([128, 512], f32, tag="mmps",
                                            name="x2_ps")
                        x2_ps = x2_ps2[:]
                        for hc in range(NHB):
                            nc.tensor.matmul(x2_ps,
                                             hT_sb[:, hc, tt * 128:(tt + 1) * 128],
                                             fc2w_sb[:, hc, :],
                                             start=(hc == 0), stop=(hc == NHB - 1))
                        xn = xn_all[:, tt, :]
                        nc.vector.tensor_add(xn, x2_ps, y_all[:, tt, :])
                        if not trivial:
                            nc.vector.tensor_add(xn, xn, fc2b_bc[:])
                        layer_norm(xn)
                        if l == L - 1:
                            acc = small.tile([128, OUT], f32, tag="acc", name="acc")
                            for o in range(OUT):
                                wro = work.tile([128, E], f32, tag="wro", bufs=2,
                                                name="wro")
                                nc.sync.dma_start(
                                    wro[:], outw_d[o, ht * 128:(ht + 1) * 128, :])
                                prod = work.tile([128, E], f32, tag="prod", bufs=2,
                                                 name="prod")
                                nc.vector.tensor_mul(prod[:], xn, wro[:])
                                nc.vector.reduce_sum(acc[:, o:o + 1], prod[:],
                                                     axis=mybir.AxisListType.X)
                            nc.sync.dma_start(headp_d[tt], acc[:])
                        else:
                            for ec in range(4):
                                trp = ps_tr.tile([128, 128], f32, tag="trps",
                                                 name="trp")
                                nc.tensor.transpose(trp[:],
                                                    xn[:, ec * 128:(ec + 1) * 128],
                                                    ident[:])
                                nc.any.tensor_copy(
                                    xTc_sb[:, ec, tt * 128:(tt + 1) * 128], trp[:])
                    if l < L - 1:
                        ag_in = dram.tile([E, HALF], bf16, tag=f"ag_in{b}",
                                          name="ag_in")
                        for ec in range(4):
                            nc.sync.dma_start(
                                ag_in[ec * 128:(ec + 1) * 128, :],
                                xTc_sb[:, ec, b * 256:(b + 1) * 256])
                        ag_out = dram.tile([BS, HALF], bf16, tag=f"ag_out{b}",
                                           addr_space="Local" if timing
                                           else "Shared",
                                           name="ag_out")
                        if timing:
                            nc.sync.dma_start(ag_out[0:8, :], ag_in[0:8, :])
                        else:
                            nc.gpsimd.collective_compute(
                                "AllGather", ALU.bypass,
                                replica_groups=[list(range(NC))],
                                ins=[ag_in.opt()], outs=[ag_out.opt()],
                            )
                        xsrc_holder[b] = ag_out[:]

                # emission order = per-engine program order: each batch's
                # attention runs back-to-back with the other batch's
                # collectives + LN/FFN so PE never waits on the network.
                build_stacks(0)
                attention(0)
                build_stacks(1)
                attention(1)
                halfpipe(0)
                tailpipe(0)
                halfpipe(1)
                tailpipe(1)

    nc.compile()
    return nc


# ---------------------------------------------------------------------------
# host side
# ---------------------------------------------------------------------------
_STATE: dict = {}


def _pos_encoding_np():
    pos = np.arange(S, dtype=np.float32)[:, None]
    div = np.exp(np.arange(0, E, 2, dtype=np.float32) * (-np.log(10000.0) / E))
    pe = np.zeros((S, E), np.float32)
    pe[:, 0::2] = np.sin(pos * div)
    pe[:, 1::2] = np.cos(pos * div)
    return pe


def _bf(x):
    return np.ascontiguousarray(np.asarray(x, np.float32).astype(ml_dtypes.bfloat16))


def _f32(x):
    return np.ascontiguousarray(np.asarray(x, np.float32))


def kernel(inputs, emb, ln_w, ln_b, q_w, q_b, k_w, k_b, v_w, v_b,
           fc1_w, fc1_b, fc2_w, fc2_b, out_w, out_b):
    idx = np.asarray(inputs)
    emb = _f32(emb)
    x0 = emb[idx.reshape(-1)] + np.tile(_pos_encoding_np(), (B, 1))  # [BS, E]
    # per-batch xsrc layout: row c*512+e; col j -> batch-b token c*256+j
    x0_b = x0.reshape(B, NC, HALF, E).transpose(0, 1, 3, 2)  # [B, NC, E, HALF]
    xsrc = [np.ascontiguousarray(x0_b[b].reshape(BS, HALF)) for b in range(B)]

    trivial = bool(
        np.all(np.asarray(ln_w, np.float32) == 1.0)
        and np.all(np.asarray(ln_b, np.float32) == 0.0)
        and np.all(np.asarray(q_b, np.float32) == 0.0)
        and np.all(np.asarray(k_b, np.float32) == 0.0)
        and np.all(np.asarray(v_b, np.float32) == 0.0)
        and np.all(np.asarray(fc1_b, np.float32) == 0.0)
        and np.all(np.asarray(fc2_b, np.float32) == 0.0))
    key = ("nc", trivial)
    if key not in _STATE:
        _STATE[key] = build_nc(trivial=trivial)
    nc = _STATE[key]

    q_w, k_w, v_w = _f32(q_w), _f32(k_w), _f32(v_w)
    fc1_w, fc2_w = _f32(fc1_w), _f32(fc2_w)
    out_w = _f32(out_w)
    Wr = out_w.reshape(S, E, OUT)

    fc1_pack = _bf(fc1_w.reshape(4, 128, HID).transpose(1, 0, 2))
    fc1b_pack = _f32(np.asarray(fc1_b, np.float32).reshape(NHB, 128).T)
    fc2_pack = _bf(fc2_w.reshape(NHB, 128, E).transpose(1, 0, 2))

    in_maps = []
    for c in range(NC):
        hs = slice(c * DH, (c + 1) * DH)
        qk = np.concatenate([q_w[:, hs], k_w[:, hs]], axis=1)  # [E, 128]
        in_maps.append({
            "xsrc0": _bf(xsrc[0]),
            "xsrc1": _bf(xsrc[1]),
            "qkw": _bf(qk.reshape(4, 128, 128).transpose(1, 0, 2)),
            "vw": _bf(v_w[:, hs].reshape(4, 128, DH).transpose(1, 0, 2)),
            "qkb": _f32(np.concatenate([np.asarray(q_b, np.float32)[hs],
                                        np.asarray(k_b, np.float32)[hs]])[:, None]),
            "vb": _f32(np.asarray(v_b, np.float32)[hs][:, None]),
            "fc1w": fc1_pack,
            "fc1b": fc1b_pack,
            "fc2w": fc2_pack,
            "fc2b": _f32(fc2_b),
            "lnw": _f32(ln_w),
            "lnb": _f32(ln_b),
            "outw": _f32(Wr[c * HALF:(c + 1) * HALF].transpose(2, 0, 1)),
        })

    res = run_bass_kernel_spmd(nc, in_maps, core_ids=list(range(NC)))
    _STATE["last_results"] = res

    out = np.zeros((B, OUT), np.float64)
    for c in range(NC):
        hp = res.results[c]["headp"]  # [4,128,OUT]; tt 0,1 -> batch0, 2,3 -> batch1
        out[0] += hp[0:2].sum(axis=(0, 1), dtype=np.float64)
        out[1] += hp[2:4].sum(axis=(0, 1), dtype=np.float64)
    out += np.asarray(out_b, np.float32)[None, :].astype(np.float64)
    return out.astype(np.float32)
